# revision 9
# baseline (speedup 1.0000x reference)
"""Trainium2 Bass kernel: CRF loss (nn_CRF_60112362275454).

Strategy (data-parallel over batch, 8 cores x 8 batch elems):
  - emit^T[tag, (s,b)] = Wp^T @ features^T via PE, K=1024 tiled by 128.
  - Forward recurrence in LINEAR space: P_t = E^T (P_{t-1} * exp(emit_t))
    with E = exp(transitions); constant renorm P *= 2^-52 every 8 steps
    (exact power of two; fp32 range validated offline: |P| <= ~1e16).
  - Tag axis permuted (0<->1) so EOS lands on partition 0: the per-step
    EOS-row snapshot (ACT copy) and all final assembly stay on partition 0.
  - Gold path: host-prepared one-hot/count masks (index preprocessing of
    int inputs only); all f32 FLOPs on device (tensor_tensor_reduce).
  - Each core emits a partial loss scalar; host sums the 8 partials.
"""
import numpy as np
from contextlib import ExitStack

import concourse.bass as bass
import concourse.mybir as mybir
import concourse.tile as tile
from concourse.bass_utils import run_bass_kernel_spmd

S, B, D, T = 256, 64, 1024, 64
BOS, EOS, PAD = 0, 1, 2
NCORES = 8
BS = B // NCORES          # 8 batch elems per core
SB = S * BS               # 2048 (s,b) columns per core
R = 8                     # renorm cadence (steps)
RENORM = 2.0 ** -52       # exact power-of-two rescale
C_LOG = 52 * float(np.log(2.0))
NCHAINS = 2               # independent scan chains (latency hiding)
CW = BS // NCHAINS
KT = D // 128             # 8 K-tiles
NCHUNK = 4                # emit column chunks
CHUNK = SB // NCHUNK      # 512

F32 = mybir.dt.float32
AF = mybir.ActivationFunctionType
ALU = mybir.AluOpType


def _build_nc():
    nc = bass.Bass()
    feat = nc.dram_tensor("feat", [S, BS, D], F32, kind="ExternalInput")
    wt = nc.dram_tensor("wt", [D, T], F32, kind="ExternalInput")
    bias = nc.dram_tensor("bias", [T, 1], F32, kind="ExternalInput")
    transp = nc.dram_tensor("transp", [T, T], F32, kind="ExternalInput")
    gmask = nc.dram_tensor("gmask", [T, SB], F32, kind="ExternalInput")
    c64 = nc.dram_tensor("c64", [T, T], F32, kind="ExternalInput")
    gcount = nc.dram_tensor("gcount", [T, 1], F32, kind="ExternalInput")
    pickmask = nc.dram_tensor("pickmask", [1, SB], F32, kind="ExternalInput")
    cw = nc.dram_tensor("cw", [1, BS], F32, kind="ExternalInput")
    out = nc.dram_tensor("out", [1, 1], F32, kind="ExternalOutput")

    # features viewed as [ktile, p=128(d), s, b] for d-on-partitions DMA
    featv = feat.rearrange("s b (k p) -> k p s b", p=128)

    with tile.TileContext(nc) as tc, ExitStack() as ctx:
        consts = ctx.enter_context(tc.tile_pool(name="consts", bufs=1))
        featp = ctx.enter_context(tc.tile_pool(name="featp", bufs=4))
        qp = ctx.enter_context(tc.tile_pool(name="qp", bufs=4))
        scratchp = ctx.enter_context(tc.tile_pool(name="scratch", bufs=2))
        emitp = ctx.enter_context(tc.tile_pool(name="emitp", bufs=2, space="PSUM"))
        scanp = ctx.enter_context(tc.tile_pool(name="scanp", bufs=2, space="PSUM"))
        smallp = ctx.enter_context(tc.tile_pool(name="smallp", bufs=1, space="PSUM"))

        # ---- constants in ----
        wt_sb = consts.tile([128, KT * T], F32, tag="wt")
        for k in range(KT):
            nc.sync.dma_start(wt_sb[:, k * T:(k + 1) * T], wt[k * 128:(k + 1) * 128, :])
        b_sb = consts.tile([T, 1], F32, tag="bias")
        nc.sync.dma_start(b_sb[:], bias[:, :])
        tr_sb = consts.tile([T, T], F32, tag="tr")
        nc.sync.dma_start(tr_sb[:], transp[:, :])
        gm_sb = consts.tile([T, SB], F32, tag="gmask")
        nc.sync.dma_start(gm_sb[:], gmask[:, :])
        c64_sb = consts.tile([T, T], F32, tag="c64")
        nc.sync.dma_start(c64_sb[:], c64[:, :])
        gc_sb = consts.tile([T, 1], F32, tag="gcount")
        nc.sync.dma_start(gc_sb[:], gcount[:, :])
        pm_sb = consts.tile([1, SB], F32, tag="pickmask")
        nc.sync.dma_start(pm_sb[:], pickmask[:, :])
        cw_sb = consts.tile([1, BS], F32, tag="cw")
        nc.sync.dma_start(cw_sb[:], cw[:, :])

        E_sb = consts.tile([T, T], F32, tag="E")
        nc.scalar.activation(E_sb[:], tr_sb[:], AF.Exp)
        ones_sb = consts.tile([T, 1], F32, tag="ones")
        nc.vector.memset(ones_sb[:], 1.0)

        # ---- emit matmul + exp + gold-emit partials ----
        expemit = consts.tile([T, SB], F32, tag="expemit")
        goldpart = consts.tile([T, 6], F32, tag="goldpart")
        for n in range(NCHUNK):
            ps = emitp.tile([T, CHUNK], F32, tag="emit")
            for k in range(KT):
                ft = featp.tile([128, CHUNK], F32, tag="feat")
                nc.sync.dma_start(ft[:], featv[k, :, n * 64:(n + 1) * 64, :])
                nc.tensor.matmul(ps[:], wt_sb[:, k * T:(k + 1) * T], ft[:],
                                 start=(k == 0), stop=(k == KT - 1))
            nc.scalar.activation(expemit[:, n * CHUNK:(n + 1) * CHUNK], ps[:],
                                 AF.Exp, bias=b_sb[:, 0:1])
            sc = scratchp.tile([T, CHUNK], F32, tag="sc")
            nc.vector.tensor_mul(sc[:], ps[:], gm_sb[:, n * CHUNK:(n + 1) * CHUNK])
            nc.vector.reduce_sum(goldpart[:, n:n + 1], sc[:],
                                 axis=mybir.AxisListType.X)

        # ---- gold: transitions & bias terms ----
        sc64 = scratchp.tile([T, T], F32, tag="sc64")
        nc.vector.tensor_mul(sc64[:], tr_sb[:], c64_sb[:])
        nc.vector.reduce_sum(goldpart[:, 4:5], sc64[:], axis=mybir.AxisListType.X)
        nc.vector.tensor_mul(goldpart[:, 5:6], b_sb[:], gc_sb[:])
        goldvec = consts.tile([T, 1], F32, tag="goldvec")
        nc.vector.reduce_sum(goldvec[:], goldpart[:], axis=mybir.AxisListType.X)
        gold_ps = smallp.tile([1, 1], F32, tag="goldtot")
        nc.tensor.matmul(gold_ps[:], ones_sb[:], goldvec[:], start=True, stop=True)

        # ---- scan ----
        hist = consts.tile([1, SB], F32, tag="hist")
        nc.vector.memset(hist[0:1, 0:BS], 1.0)  # t=0 slots never picked; avoid NaN*0
        BOSP = 32  # permuted BOS partition (matmul base partition must be 0/32/64)
        prev = []
        for c in range(NCHAINS):
            p0 = scanp.tile([T, CW], F32, tag=f"scan{c}")
            nc.tensor.matmul(p0[:], E_sb[BOSP:BOSP + 1, :],
                             expemit[BOSP:BOSP + 1, c * CW:(c + 1) * CW],
                             start=True, stop=True)
            prev.append(p0)
        for t in range(1, S):
            scaled = (t > 1) and ((t - 1) % R == 0)
            for c in range(NCHAINS):
                lo = t * BS + c * CW
                q = qp.tile([T, CW], F32, tag=f"q{c}")
                if scaled:
                    nc.vector.tensor_scalar_mul(prev[c][:], prev[c][:], RENORM)
                nc.vector.tensor_mul(q[:], prev[c][:], expemit[:, lo:lo + CW])
                ns = scanp.tile([T, CW], F32, tag=f"scan{c}")
                nc.tensor.matmul(ns[:], E_sb[:], q[:], start=True, stop=True)
                nc.scalar.activation(hist[0:1, lo:lo + CW], ns[0:1, :], AF.Copy)
                prev[c] = ns

        # ---- final assembly (partition 0) ----
        pmul = consts.tile([1, SB], F32, tag="pmul")
        nc.vector.tensor_mul(pmul[:], hist[:], pm_sb[:])
        pick8 = consts.tile([1, BS], F32, tag="pick8")
        nc.vector.reduce_sum(pick8[:], pmul[0:1, :].rearrange("p (t b) -> p b t", b=BS),
                             axis=mybir.AxisListType.X)
        zrow = consts.tile([1, BS], F32, tag="zrow")
        nc.scalar.activation(zrow[:], pick8[:], AF.Ln)
        z2 = consts.tile([1, BS], F32, tag="z2")
        nc.vector.tensor_add(z2[:], zrow[:], cw_sb[:])
        zsum = consts.tile([1, 1], F32, tag="zsum")
        nc.vector.reduce_sum(zsum[:], z2[:], axis=mybir.AxisListType.X)
        lossp = consts.tile([1, 1], F32, tag="lossp")
        nc.vector.tensor_sub(lossp[:], zsum[:], gold_ps[:])
        nc.sync.dma_start(out[:, :], lossp[:])

    # Raw Bass under TileContext skips two bacc legalization passes the NEFF
    # compiler requires: populating .instr bytes for extended-ISA insts, and
    # splitting >2 on_wait entries onto InstEventSemaphore (walrus rejects
    # "Too many sync wait commands" otherwise).
    mybir.codegen_inst_isa_subclasses(nc)
    import bass_rust
    bass_rust.generate_event_semaphores(nc)
    return nc


_CACHE = {}


def _get_nc():
    if "nc" not in _CACHE:
        _CACHE["nc"] = _build_nc()
    return _CACHE["nc"]


def _host_prep(features, tags, seq_lens, W, b, transitions):
    features = np.ascontiguousarray(np.asarray(features, dtype=np.float32))
    tags = np.asarray(tags).astype(np.int64)
    seq_lens = np.asarray(seq_lens).astype(np.int64)
    W = np.asarray(W, dtype=np.float32)
    bvec = np.asarray(b, dtype=np.float32)
    transitions = np.asarray(transitions, dtype=np.float32)

    # tag permutation sigma(old)=new: EOS->0 (hist snapshots on partition 0),
    # BOS->32 (matmul base-partition constraint), 3-cycle 0->32->1->0.
    sigma = np.arange(T)
    sigma[EOS], sigma[BOS], sigma[32] = 0, 32, 1
    inv = np.argsort(sigma)
    Wt_p = np.ascontiguousarray(W[inv, :].T)                   # [D, T]
    b_p = np.ascontiguousarray(bvec[inv].reshape(T, 1))
    trans_p = np.ascontiguousarray(transitions[np.ix_(inv, inv)])

    pad_row = np.full((1, B), PAD, tags.dtype)
    nxt = np.concatenate([tags[1:], pad_row], axis=0)
    active = np.arange(S)[:, None] < seq_lens[None, :]          # s <= len-1
    tstar = seq_lens - 1
    wnum = (seq_lens - 2) // R

    in_maps = []
    for c in range(NCORES):
        bsl = slice(c * BS, (c + 1) * BS)
        f_c = np.ascontiguousarray(features[:, bsl, :])
        tg = tags[:, bsl]
        nx = nxt[:, bsl]
        act = active[:, bsl].astype(np.float32)
        gm = np.zeros((T, SB), np.float32)
        cols = np.arange(SB).reshape(S, BS)
        gm[sigma[tg].ravel(), cols.ravel()] = act.ravel()
        c64m = np.zeros((T, T), np.float32)
        np.add.at(c64m, (sigma[tg].ravel(), sigma[nx].ravel()), act.ravel())
        gc = gm.sum(axis=1).reshape(T, 1).astype(np.float32)
        pm = np.zeros((1, SB), np.float32)
        pm[0, tstar[bsl] * BS + np.arange(BS)] = 1.0
        cwv = (wnum[bsl].astype(np.float64) * C_LOG).astype(np.float32).reshape(1, BS)
        in_maps.append({
            "feat": f_c, "wt": Wt_p, "bias": b_p, "transp": trans_p,
            "gmask": gm, "c64": c64m, "gcount": gc, "pickmask": pm, "cw": cwv,
        })
    return in_maps


def kernel(features, tags, seq_lens, W, b, transitions):
    in_maps = _host_prep(features, tags, seq_lens, W, b, transitions)
    nc = _get_nc()
    res = run_bass_kernel_spmd(nc, in_maps, list(range(NCORES)))
    total = np.float64(0.0)
    for r in res.results:
        total += np.float64(np.asarray(r["out"]).reshape(-1)[0])
    return np.array(total, dtype=np.float32)


# revision 24
# speedup vs baseline: 4.7353x; 4.7353x over previous
"""Trainium2 Bass kernel: CRF loss (nn_CRF_60112362275454).

Strategy (data-parallel over batch, 8 cores x 8 batch elems):
  - emit^T[tag, (s,b)] = Wp^T @ features^T via PE, K=1024 tiled by 128.
  - Forward recurrence in LINEAR space: P_t = E^T (P_{t-1} * exp(emit_t))
    with E = exp(transitions); constant renorm P *= 2^-52 every 8 steps
    (exact power of two; fp32 range validated offline: |P| <= ~1e16).
  - Tag axis permuted (0<->1) so EOS lands on partition 0: the per-step
    EOS-row snapshot (ACT copy) and all final assembly stay on partition 0.
  - Gold path: host-prepared one-hot/count masks (index preprocessing of
    int inputs only); all f32 FLOPs on device (tensor_tensor_reduce).
  - Each core emits a partial loss scalar; host sums the 8 partials.
"""
import numpy as np
from contextlib import ExitStack

import concourse.bass as bass
import concourse.bass_isa as bass_isa
import concourse.mybir as mybir
import concourse.tile as tile
from concourse.bass_utils import run_bass_kernel_spmd

S, B, D, T = 256, 64, 1024, 64
BOS, EOS, PAD = 0, 1, 2
NCORES = 8
BS = B // NCORES          # 8 batch elems per core
SB = S * BS               # 2048 (s,b) columns per core
R = 8                     # renorm cadence (steps)
RENORM = 2.0 ** -52       # exact power-of-two rescale
C_LOG = 52 * float(np.log(2.0))
NCHAINS = 2               # independent scan chains (latency hiding)
CW = BS // NCHAINS
KT = D // 128             # 8 K-tiles
NCHUNK = 4                # emit column chunks
CHUNK = SB // NCHUNK      # 512

F32 = mybir.dt.float32
AF = mybir.ActivationFunctionType
ALU = mybir.AluOpType


def _build_nc():
    nc = bass.Bass()
    # feat arrives host-transposed to [D, S*BS] so each SBUF partition row
    # is an 8KB contiguous HBM run (DMA packet efficiency).
    feat = nc.dram_tensor("feat", [D, SB], F32, kind="ExternalInput")
    wt = nc.dram_tensor("wt", [D, T], F32, kind="ExternalInput")
    bias = nc.dram_tensor("bias", [T, 1], F32, kind="ExternalInput")
    transp = nc.dram_tensor("transp", [T, T], F32, kind="ExternalInput")
    gmask = nc.dram_tensor("gmask", [T, SB], F32, kind="ExternalInput")
    c64 = nc.dram_tensor("c64", [T, T], F32, kind="ExternalInput")
    gcount = nc.dram_tensor("gcount", [T, 1], F32, kind="ExternalInput")
    pickmask = nc.dram_tensor("pickmask", [1, SB], F32, kind="ExternalInput")
    cw = nc.dram_tensor("cw", [1, BS], F32, kind="ExternalInput")
    out = nc.dram_tensor("out", [1, 1], F32, kind="ExternalOutput")

    with tile.TileContext(nc) as tc, ExitStack() as ctx:
        consts = ctx.enter_context(tc.tile_pool(name="consts", bufs=1))
        featp = ctx.enter_context(tc.tile_pool(name="featp", bufs=2))
        qp = ctx.enter_context(tc.tile_pool(name="qp", bufs=4))
        scratchp = ctx.enter_context(tc.tile_pool(name="scratch", bufs=2))
        emitp = ctx.enter_context(tc.tile_pool(name="emitp", bufs=1, space="PSUM"))
        scanp = ctx.enter_context(tc.tile_pool(name="scanp", bufs=2, space="PSUM"))

        # ---- constants in ----
        wt_sb = consts.tile([128, KT * T], F32, tag="wt")
        for k in range(KT):
            nc.sync.dma_start(wt_sb[:, k * T:(k + 1) * T], wt[k * 128:(k + 1) * 128, :])
        b_sb = consts.tile([T, 1], F32, tag="bias")
        nc.sync.dma_start(b_sb[:], bias[:, :])
        tr_sb = consts.tile([T, T], F32, tag="tr")
        nc.sync.dma_start(tr_sb[:], transp[:, :])
        gm_sb = consts.tile([T, SB], F32, tag="gmask")
        nc.sync.dma_start(gm_sb[:], gmask[:, :])
        c64_sb = consts.tile([T, T], F32, tag="c64")
        nc.sync.dma_start(c64_sb[:], c64[:, :])
        gc_sb = consts.tile([T, 1], F32, tag="gcount")
        nc.sync.dma_start(gc_sb[:], gcount[:, :])
        pm_sb = consts.tile([1, SB], F32, tag="pickmask")
        nc.sync.dma_start(pm_sb[:], pickmask[:, :])
        cw_sb = consts.tile([1, BS], F32, tag="cw")
        nc.sync.dma_start(cw_sb[:], cw[:, :])

        E_sb = consts.tile([T, T], F32, tag="E")
        nc.scalar.activation(E_sb[:], tr_sb[:], AF.Exp)
        ones_sb = consts.tile([T, 1], F32, tag="ones")
        nc.vector.memset(ones_sb[:], 1.0)

        # ---- emit matmul + exp + gold-emit partials ----
        # k outer so each feat k-tile is one 1MB contiguous DMA; all 4
        # emit column-chunks accumulate simultaneously in PSUM (4 banks).
        expemit = consts.tile([T, SB], F32, tag="expemit")
        goldpart = consts.tile([T, 6], F32, tag="goldpart")
        emits = [emitp.tile([T, CHUNK], F32, tag=f"emit{n}", name=f"emit{n}")
                 for n in range(NCHUNK)]
        for k in range(KT):
            ft = featp.tile([128, SB], F32, tag="feat")
            nc.sync.dma_start(ft[:], feat[k * 128:(k + 1) * 128, :])
            for n in range(NCHUNK):
                nc.tensor.matmul(emits[n][:], wt_sb[:, k * T:(k + 1) * T],
                                 ft[:, n * CHUNK:(n + 1) * CHUNK],
                                 start=(k == 0), stop=(k == KT - 1))
        for n in range(NCHUNK):
            ps = emits[n]
            nc.scalar.activation(expemit[:, n * CHUNK:(n + 1) * CHUNK], ps[:],
                                 AF.Exp, bias=b_sb[:, 0:1])
            sc = scratchp.tile([T, CHUNK], F32, tag="sc")
            nc.vector.tensor_mul(sc[:], ps[:], gm_sb[:, n * CHUNK:(n + 1) * CHUNK])
            nc.vector.reduce_sum(goldpart[:, n:n + 1], sc[:],
                                 axis=mybir.AxisListType.X)

        # ---- gold: transitions & bias terms ----
        sc64 = scratchp.tile([T, T], F32, tag="sc64")
        nc.vector.tensor_mul(sc64[:], tr_sb[:], c64_sb[:])
        nc.vector.reduce_sum(goldpart[:, 4:5], sc64[:], axis=mybir.AxisListType.X)
        nc.vector.tensor_mul(goldpart[:, 5:6], b_sb[:], gc_sb[:])
        goldvec = consts.tile([T, 1], F32, tag="goldvec")
        nc.vector.reduce_sum(goldvec[:], goldpart[:], axis=mybir.AxisListType.X)
        # cross-partition sum via ones-matmul; reuse an emit PSUM slot (freed
        # after its exp/gold consumers) to stay within 8 banks
        gold_ps = emitp.tile([1, 1], F32, tag="emit0", name="gold_ps")
        nc.tensor.matmul(gold_ps[:], ones_sb[:], goldvec[:], start=True, stop=True)

        # ---- scan ----
        hist = consts.tile([1, SB], F32, tag="hist")
        nc.vector.memset(hist[0:1, 0:BS], 1.0)  # t=0 slots never picked; avoid NaN*0
        BOSP = 32  # permuted BOS partition (matmul base partition must be 0/32/64)
        prev = []
        for c in range(NCHAINS):
            p0 = scanp.tile([T, CW], F32, tag=f"scan{c}")
            nc.tensor.matmul(p0[:], E_sb[BOSP:BOSP + 1, :],
                             expemit[BOSP:BOSP + 1, c * CW:(c + 1) * CW],
                             start=True, stop=True)
            prev.append(p0)
        for t in range(1, S):
            scaled = (t > 1) and ((t - 1) % R == 0)
            for c in range(NCHAINS):
                lo = t * BS + c * CW
                q = qp.tile([T, CW], F32, tag=f"q{c}")
                if scaled:
                    nc.vector.tensor_scalar_mul(prev[c][:], prev[c][:], RENORM)
                nc.vector.tensor_mul(q[:], prev[c][:], expemit[:, lo:lo + CW])
                ns = scanp.tile([T, CW], F32, tag=f"scan{c}")
                nc.tensor.matmul(ns[:], E_sb[:], q[:], start=True, stop=True)
                nc.scalar.activation(hist[0:1, lo:lo + CW], ns[0:1, :], AF.Copy)
                prev[c] = ns

        # ---- final assembly (partition 0) ----
        pmul = consts.tile([1, SB], F32, tag="pmul")
        nc.vector.tensor_mul(pmul[:], hist[:], pm_sb[:])
        pick8 = consts.tile([1, BS], F32, tag="pick8")
        nc.vector.reduce_sum(pick8[:], pmul[0:1, :].rearrange("p (t b) -> p b t", b=BS),
                             axis=mybir.AxisListType.X)
        zrow = consts.tile([1, BS], F32, tag="zrow")
        nc.scalar.activation(zrow[:], pick8[:], AF.Ln)
        z2 = consts.tile([1, BS], F32, tag="z2")
        nc.vector.tensor_add(z2[:], zrow[:], cw_sb[:])
        zsum = consts.tile([1, 1], F32, tag="zsum")
        nc.vector.reduce_sum(zsum[:], z2[:], axis=mybir.AxisListType.X)
        lossp = consts.tile([1, 1], F32, tag="lossp")
        nc.vector.tensor_sub(lossp[:], zsum[:], gold_ps[:])
        nc.sync.dma_start(out[:, :], lossp[:])

    # Raw Bass under TileContext skips two bacc legalization passes the NEFF
    # compiler requires: populating .instr bytes for extended-ISA insts, and
    # splitting >2 on_wait entries onto InstEventSemaphore (walrus rejects
    # "Too many sync wait commands" otherwise).
    mybir.codegen_inst_isa_subclasses(nc)
    import bass_rust
    bass_rust.generate_event_semaphores(nc)
    return nc


_CACHE = {}


def _get_nc():
    if "nc" not in _CACHE:
        _CACHE["nc"] = _build_nc()
    return _CACHE["nc"]


def _host_prep(features, tags, seq_lens, W, b, transitions):
    features = np.ascontiguousarray(np.asarray(features, dtype=np.float32))
    tags = np.asarray(tags).astype(np.int64)
    seq_lens = np.asarray(seq_lens).astype(np.int64)
    W = np.asarray(W, dtype=np.float32)
    bvec = np.asarray(b, dtype=np.float32)
    transitions = np.asarray(transitions, dtype=np.float32)

    # tag permutation sigma(old)=new: EOS->0 (hist snapshots on partition 0),
    # BOS->32 (matmul base-partition constraint), 3-cycle 0->32->1->0.
    sigma = np.arange(T)
    sigma[EOS], sigma[BOS], sigma[32] = 0, 32, 1
    inv = np.argsort(sigma)
    Wt_p = np.ascontiguousarray(W[inv, :].T)                   # [D, T]
    b_p = np.ascontiguousarray(bvec[inv].reshape(T, 1))
    trans_p = np.ascontiguousarray(transitions[np.ix_(inv, inv)])

    pad_row = np.full((1, B), PAD, tags.dtype)
    nxt = np.concatenate([tags[1:], pad_row], axis=0)
    active = np.arange(S)[:, None] < seq_lens[None, :]          # s <= len-1
    tstar = seq_lens - 1
    wnum = (seq_lens - 2) // R

    in_maps = []
    for c in range(NCORES):
        bsl = slice(c * BS, (c + 1) * BS)
        # [S, BS, D] -> [D, S*BS] host transpose (DMA layout prep)
        f_c = np.ascontiguousarray(
            features[:, bsl, :].transpose(2, 0, 1).reshape(D, SB))
        tg = tags[:, bsl]
        nx = nxt[:, bsl]
        act = active[:, bsl].astype(np.float32)
        gm = np.zeros((T, SB), np.float32)
        cols = np.arange(SB).reshape(S, BS)
        gm[sigma[tg].ravel(), cols.ravel()] = act.ravel()
        c64m = np.zeros((T, T), np.float32)
        np.add.at(c64m, (sigma[tg].ravel(), sigma[nx].ravel()), act.ravel())
        gc = gm.sum(axis=1).reshape(T, 1).astype(np.float32)
        pm = np.zeros((1, SB), np.float32)
        pm[0, tstar[bsl] * BS + np.arange(BS)] = 1.0
        cwv = (wnum[bsl].astype(np.float64) * C_LOG).astype(np.float32).reshape(1, BS)
        in_maps.append({
            "feat": f_c, "wt": Wt_p, "bias": b_p, "transp": trans_p,
            "gmask": gm, "c64": c64m, "gcount": gc, "pickmask": pm, "cw": cwv,
        })
    return in_maps


def kernel(features, tags, seq_lens, W, b, transitions):
    in_maps = _host_prep(features, tags, seq_lens, W, b, transitions)
    nc = _get_nc()
    res = run_bass_kernel_spmd(nc, in_maps, list(range(NCORES)))
    total = np.float64(0.0)
    for r in res.results:
        total += np.float64(np.asarray(r["out"]).reshape(-1)[0])
    return np.array(total, dtype=np.float32)
